# revision 24
# baseline (speedup 1.0000x reference)
"""CBOW negative-sampling loss on 8 Trainium2 NeuronCores.

Strategy: replicate the embedding tables, data-parallel over the batch dim.
Each core handles 2048 of the 16384 batch rows.

Host side: u_emb (x1024), w_emb (x32) and -w_emb are concatenated into one
[3V, D] fp8-e4m3 table (neg_w indices offset by +V, pos_w by +2V into the
negated copy, so every score product already carries its loss sign and the
dot reduce needs no sign handling). The gather traffic is descriptor-rate-
bound (~10ns/row/engine), so the layout splits u and w gathers:
  - u-rows are gathered raw fp8 (no cast) and consumed by the TensorEngine,
    which handles fp8 natively;
  - w-rows are gathered with an on-the-fly SWDGE cast to bf16 so the DVE
    keeps its 2x 16-bit rate for the score products.
The scale factors keep fp8 values in the normal range and are divided back
out in the activation's scale factor.

Per-core kernel layout:
  - batch row b -> chunk c = b // 128, partition p = b % 128.
  - 16 chunks in ramped groups. Per group one u-gather (k-major slabs: all
    k=0 rows, then k=1, ...) and one w-gather; w0+w1 share one gather and
    the last group uses a single combined cast gather. Transfer order is
    tuned so the first w block lands right after u0 (DVE starts early).
  - h = sum of the 8 context embeddings: 8 accumulating identity-weight
    matmuls per group (rhs = the k-th fp8 slab) on the TensorEngine ->
    PSUM f32, then one ACT copy PSUM -> SBUF bf16. The last (1-chunk)
    group sums on DVE instead to shorten the tail chain.
  - dots on DVE: one broadcast-mult (2x bf16 rate), 3 contiguous fold-adds
    128->16, one TensorReduce -> softplus(+x) terms for every column.
  - the TensorEngine's HAM activity window is kept hot with filler matmuls
    so the h-sum runs at 2.4 GHz instead of the cold 1.2 GHz.
  - softplus per group on ACT: Exp then Ln(1+x) with accum_out -> lp[:, g].
    Both functions come from one activation table (id 6) preloaded at
    kernel start, so no mid-kernel table swaps.
  - finale: reduce lp rows, PE ones-matmul to collapse partitions -> [1,1],
    host sums the 8 per-core scalars.

loss = sum_b softplus(-score_b) + sum_{b,k} softplus(+neg_score_bk)
"""

import sys

import numpy as np

sys.path.insert(0, "/opt/trn_rl_repo")

from concourse import bacc, bass, mybir, tile  # noqa: E402
from concourse.bass_utils import run_bass_kernel_spmd  # noqa: E402
from concourse.masks import make_identity  # noqa: E402

V, D = 100000, 128
B, C, K = 16384, 8, 5
N_CORES = 8
P = 128
B_LOC = B // N_CORES            # 2048 batch rows per core
N_CHUNK = B_LOC // P            # 16 chunks of 128 rows
GROUPS = (1, 3, 4, 4, 3, 1)     # chunks per gather group; last is combined
assert sum(GROUPS) == N_CHUNK
# matmul moving operand and a PSUM bank both cap at 512 f32 per partition
assert max(GROUPS) * D <= 512
J = 1 + K                       # 6 w-rows per batch row (pos + negs)
R = C + J                       # 14 gathered rows per batch row
NG = len(GROUPS)
NPE = NG - 1                    # groups whose h-sum runs on the PE
STARTS = [sum(GROUPS[:g]) for g in range(NG)]
# gidx column layout: [u0k .. u4k | w0 .. w4 | u5k w5]
U_OFF = [sum(C * n for n in GROUPS[:g]) for g in range(NPE)]
W_BASE = sum(C * n for n in GROUPS[:NPE])
W_OFF = [W_BASE + sum(J * n for n in GROUPS[:g]) for g in range(NPE)]
G5_OFF = W_BASE + sum(J * n for n in GROUPS[:NPE])
assert G5_OFF + R * GROUPS[NPE] == N_CHUNK * R

S_U = 1024.0                    # host-side scale into fp8 normal range
S_W = 32.0
ACT_SCALE = 1.0 / (S_U * S_W)   # divided back out inside the Exp

_NC_CACHE = {}


def _build_bass():
    nc = bacc.Bacc(
        "TRN2",
        target_bir_lowering=False,
        debug=False,
        dynamic_dma_scratch_size=65536,
    )

    bf16 = mybir.dt.bfloat16
    fp32 = mybir.dt.float32
    fp8 = mybir.dt.float8e4
    X = mybir.AxisListType.X
    ADD = mybir.AluOpType.add
    MUL = mybir.AluOpType.mult
    EXP = mybir.ActivationFunctionType.Exp
    LN = mybir.ActivationFunctionType.Ln
    COPY = mybir.ActivationFunctionType.Copy

    emb = nc.dram_tensor("emb_cat", [3 * V, D], fp8, kind="ExternalInput")
    gidx = nc.dram_tensor(
        "gidx", [P, N_CHUNK * R], mybir.dt.int32, kind="ExternalInput"
    )
    loss = nc.dram_tensor("loss_part", [1, 1], fp32, kind="ExternalOutput")

    with tile.TileContext(nc) as tc:
        with (
            tc.tile_pool(name="idx", bufs=1) as idx_pool,
            tc.tile_pool(name="ub", bufs=5) as ub_pool,
            tc.tile_pool(name="wb", bufs=5) as wb_pool,
            tc.tile_pool(name="sc", bufs=2) as sc_pool,
            tc.tile_pool(name="sp", bufs=2) as sp_pool,
            tc.tile_pool(name="fin", bufs=1) as fin_pool,
            tc.tile_pool(name="hp", bufs=3, space="PSUM") as hp_pool,
            tc.tile_pool(name="ps", bufs=1, space="PSUM") as ps_pool,
        ):
            # one table (id 6: natural_log_exp_and_others) serves Exp AND Ln
            nc.scalar.add_instruction(mybir.InstLoadActFuncSet(
                name=nc.get_next_instruction_name(), act_func_set_id=6,
                ins=[], outs=[]))

            ix_all = idx_pool.tile([P, N_CHUNK * R], mybir.dt.int32, tag="ix")
            n0 = C * GROUPS[0]   # u0 columns only: first desc-gen starts ASAP
            nc.sync.dma_start(out=ix_all[:, 0:n0], in_=gidx[:, 0:n0])
            nc.sync.dma_start(out=ix_all[:, n0:], in_=gidx[:, n0:])

            ident = fin_pool.tile([P, P], fp8, tag="ident")
            make_identity(nc, ident[:])
            ones = fin_pool.tile([P, 1], fp32, tag="ones")
            nc.gpsimd.memset(ones[:], 1.0)

            warm = ps_pool.tile([P, P], fp32, tag="warm")

            def pe_filler(count):
                # keep the TensorEngine's activity window hot; results unused
                for _ in range(count):
                    nc.tensor.matmul(out=warm[:], lhsT=ident[:],
                                     rhs=ident[:], start=True, stop=True)

            pe_filler(45)

            h_sb = fin_pool.tile([P, N_CHUNK * D], bf16, tag="h_sb")
            h16 = h_sb[:].rearrange("p (c d) -> p c d", c=N_CHUNK)
            lp = fin_pool.tile([P, NG], fp32, tag="lp")

            def gather(cols, ncols, dtype, tag, pool):
                t = pool.tile([P, ncols * D], dtype, tag=tag)
                nc.gpsimd.indirect_dma_start(
                    out=t[:],
                    out_offset=None,
                    in_=emb[:],
                    in_offset=bass.IndirectOffsetOnAxis(
                        ap=ix_all[:, cols : cols + ncols], axis=0
                    ),
                )
                return t

            # issue all gathers up front; the list order is the transfer
            # order: u0 | w0+u5+w5 (merged cast gather; the tail group's
            # data comes early since its DVE work is emitted right after
            # g0's to fill the pipeline-fill bubble) | u1 | w1 | ... | w4
            ub, wbt = {}, {}
            n5 = GROUPS[NPE]
            ub[0] = gather(U_OFF[0], C * GROUPS[0], fp8, "ub", ub_pool)
            w01 = gather(W_OFF[0], J * (GROUPS[0] + GROUPS[1]), bf16, "wb",
                         wb_pool)
            wbt[0] = w01[:, 0 : J * GROUPS[0] * D]
            wbt[1] = w01[:, J * GROUPS[0] * D :]
            m0 = gather(G5_OFF, R * n5, bf16, "g5", wb_pool)
            ub[NPE] = m0[:, 0 : C * n5 * D]
            wbt[NPE] = m0[:, C * n5 * D :]
            ub[1] = gather(U_OFF[1], C * GROUPS[1], fp8, "ub", ub_pool)
            for _g in range(2, NPE):
                ub[_g] = gather(U_OFF[_g], C * GROUPS[_g], fp8, "ub", ub_pool)
                wbt[_g] = gather(W_OFF[_g], J * GROUPS[_g], bf16, "wb",
                                 wb_pool)[:]

            for g in [0, NG - 1] + list(range(1, NPE)):
                n = GROUPS[g]
                c0 = STARTS[g]
                if g < NPE:
                    # h-sum on the TensorEngine: 8 accumulating identity
                    # matmuls (rhs = the k-th fp8 u-slab) -> PSUM f32
                    hp = hp_pool.tile([P, n * D], fp32, tag="hp")
                    for k in range(C):
                        nc.tensor.matmul(
                            out=hp[:],
                            lhsT=ident[:],
                            rhs=ub[g][:, k * n * D : (k + 1) * n * D],
                            start=(k == 0),
                            stop=(k == C - 1),
                        )
                    # PSUM f32 -> SBUF bf16 on the Scalar engine
                    nc.scalar.activation(
                        out=h_sb[:, c0 * D : (c0 + n) * D], in_=hp[:],
                        func=COPY,
                    )
                    if g + 1 < NPE:
                        pe_filler(15)
                else:
                    # tail group: binary-tree h-sum on DVE over the k-major
                    # bf16 slabs, last fold writes straight into h_sb
                    u5 = ub[g]
                    nc.vector.tensor_add(
                        out=u5[:, 0 : 4 * n * D], in0=u5[:, 0 : 4 * n * D],
                        in1=u5[:, 4 * n * D : 8 * n * D],
                    )
                    nc.vector.tensor_add(
                        out=u5[:, 0 : 2 * n * D], in0=u5[:, 0 : 2 * n * D],
                        in1=u5[:, 2 * n * D : 4 * n * D],
                    )
                    nc.vector.tensor_add(
                        out=h_sb[:, c0 * D : (c0 + n) * D],
                        in0=u5[:, 0 : n * D], in1=u5[:, n * D : 2 * n * D],
                    )

                # m[p, c, j, d] = w[p, c, j, d] * h[p, c, d], in place over w
                w4 = wbt[g].rearrange("p (c j d) -> p c j d", c=n, j=J)
                h4 = h16[:, c0 : c0 + n]
                nc.vector.tensor_mul(
                    out=w4,
                    in0=w4,
                    in1=h4[:, :, None, :].broadcast_to([P, n, J, D]),
                )
                # fold the innermost 128 -> 16 with bf16 adds before the
                # (1x-rate) TensorReduce
                for w_ in (64, 32, 16):
                    nc.vector.tensor_add(
                        out=w4[:, :, :, 0:w_],
                        in0=w4[:, :, :, 0:w_],
                        in1=w4[:, :, :, w_ : 2 * w_],
                    )
                # raw dots (f32); j=0 was sign-folded in the multiply
                sc = sc_pool.tile([P, n * J], fp32, tag="sc")
                sc3 = sc[:].rearrange("p (c j) -> p c j", j=J)
                nc.vector.tensor_reduce(
                    out=sc3, in_=w4[:, :, :, 0:16], axis=X, op=ADD,
                )
                # softplus(x) = ln(1 + exp(x)); accumulate into lp[:, g]
                sp = sp_pool.tile([P, n * J], fp32, tag="sp")
                nc.scalar.activation(out=sp[:], in_=sc[:], func=EXP,
                                     scale=ACT_SCALE)
                nc.scalar.activation(out=sp[:], in_=sp[:], func=LN, bias=1.0,
                                     accum_out=lp[:, g : g + 1])

            # per-partition loss, then collapse partitions via ones-matmul
            lp1 = fin_pool.tile([P, 1], fp32, tag="lp1")
            nc.vector.tensor_reduce(out=lp1[:], in_=lp[:], axis=X, op=ADD)
            acc = ps_pool.tile([1, 1], fp32)
            nc.tensor.matmul(out=acc[:], lhsT=ones[:], rhs=lp1[:],
                             start=True, stop=True)
            out_sb = fin_pool.tile([1, 1], fp32, tag="out")
            nc.vector.tensor_copy(out=out_sb[:], in_=acc[:])
            nc.sync.dma_start(out=loss[:], in_=out_sb[:])

    nc.compile()
    return nc


def _get_nc():
    if "nc" not in _NC_CACHE:
        _NC_CACHE["nc"] = _build_bass()
    return _NC_CACHE["nc"]


def _make_in_maps(pos_u, pos_w, neg_w, u_emb, w_emb):
    pos_u = np.asarray(pos_u).astype(np.int32)
    pos_w = np.asarray(pos_w).astype(np.int32)
    neg_w = np.asarray(neg_w).astype(np.int32)
    u_emb = np.asarray(u_emb, dtype=np.float32)
    w_emb = np.asarray(w_emb, dtype=np.float32)

    fp8_np = mybir.dt.np(mybir.dt.float8e4)
    w_s = np.clip(w_emb * S_W, -224.0, 224.0)
    emb_cat = np.ascontiguousarray(
        np.concatenate(
            [np.clip(u_emb * S_U, -224.0, 224.0), w_s, -w_s], axis=0
        ).astype(fp8_np)
    )

    in_maps = []
    for i in range(N_CORES):
        sl = slice(i * B_LOC, (i + 1) * B_LOC)
        # batch row b -> (chunk c = b // 128, partition p = b % 128)
        rows_u = pos_u[sl].reshape(N_CHUNK, P, C)          # [c, p, k]
        rows_w = np.concatenate(
            [pos_w[sl, None] + 2 * V, neg_w[sl] + V], axis=1
        ).reshape(N_CHUNK, P, J)                           # [c, p, j]
        # columns: [u0k .. u4k | w0 .. w4 | u5k w5]; u blocks k-major
        cols = []
        for g in range(NPE):
            c0, n = STARTS[g], GROUPS[g]
            cols.append(rows_u[c0 : c0 + n].transpose(1, 2, 0).reshape(P, -1))
        for g in range(NPE):
            c0, n = STARTS[g], GROUPS[g]
            cols.append(rows_w[c0 : c0 + n].transpose(1, 0, 2).reshape(P, -1))
        c0, n = STARTS[NPE], GROUPS[NPE]
        cols.append(rows_u[c0 : c0 + n].transpose(1, 2, 0).reshape(P, -1))
        cols.append(rows_w[c0 : c0 + n].transpose(1, 0, 2).reshape(P, -1))
        gidx = np.concatenate(cols, axis=1)                # [P, N_CHUNK * R]
        in_maps.append(
            {
                "emb_cat": emb_cat,
                "gidx": np.ascontiguousarray(gidx),
            }
        )
    return in_maps


def _install_axon_profile_shim():
    """Provide antenv.axon_hooks (missing in this image) so trace=True can
    capture NTFF profiles via the axon PJRT .so, and keep trace artifacts
    local instead of uploading to a bucket."""
    import contextlib
    import ctypes
    import types

    import concourse.bass_utils as bu

    bu.upload_artifacts = lambda tmpdir: tmpdir

    try:
        from antenv.axon_hooks import get_axon_ntff_profile_hook  # noqa: F401

        return
    except ImportError:
        pass

    mod = types.ModuleType("antenv.axon_hooks")
    holder = {}
    mod.set_axon_ntff_profile_hook = lambda h: holder.__setitem__("h", h)
    mod.get_axon_ntff_profile_hook = lambda: holder.get("h")
    sys.modules["antenv.axon_hooks"] = mod
    import antenv

    antenv.axon_hooks = mod

    so_path = "/opt/axon/libaxon_pjrt.so"
    lib = ctypes.CDLL(so_path)
    if not hasattr(lib, "axon_start_nrt_profile"):
        return
    lib.axon_start_nrt_profile.argtypes = [
        ctypes.POINTER(ctypes.c_int64),
        ctypes.c_size_t,
    ]
    lib.axon_start_nrt_profile.restype = ctypes.c_int64
    lib.axon_stop_nrt_profile.argtypes = [ctypes.c_char_p]
    lib.axon_stop_nrt_profile.restype = ctypes.c_int64

    @contextlib.contextmanager
    def _hook(output_dir, device_ids):
        import jax

        jax.devices()
        if device_ids:
            ids = (ctypes.c_int64 * len(device_ids))(*device_ids)
            rc = lib.axon_start_nrt_profile(ids, len(device_ids))
        else:
            rc = lib.axon_start_nrt_profile(None, 0)
        if rc != 0:
            raise RuntimeError(f"axon_start_nrt_profile rc={rc}")
        try:
            yield
        finally:
            n = lib.axon_stop_nrt_profile(str(output_dir).encode())
            print(f"profile: {n} file(s) written to {output_dir}")

    mod.set_axon_ntff_profile_hook(_hook)


def _run(in_maps, trace=False):
    if trace:
        _install_axon_profile_shim()
    nc = _get_nc()
    return run_bass_kernel_spmd(nc, in_maps, list(range(N_CORES)), trace=trace)


def kernel(pos_u, pos_w, neg_w, u_emb, w_emb):
    in_maps = _make_in_maps(pos_u, pos_w, neg_w, u_emb, w_emb)
    bkr = _run(in_maps, trace=False)
    total = 0.0
    for r in bkr.results:
        total += float(r["loss_part"].astype(np.float64).sum())
    return np.float32(total)


def kernel_traced(pos_u, pos_w, neg_w, u_emb, w_emb):
    """Like kernel() but returns (loss, BassKernelResults) with HW profile."""
    in_maps = _make_in_maps(pos_u, pos_w, neg_w, u_emb, w_emb)
    bkr = _run(in_maps, trace=True)
    total = 0.0
    for r in bkr.results:
        total += float(r["loss_part"].astype(np.float64).sum())
    return np.float32(total), bkr


# revision 25
# speedup vs baseline: 1.0161x; 1.0161x over previous
"""CBOW negative-sampling loss on 8 Trainium2 NeuronCores.

Strategy: replicate the embedding tables, data-parallel over the batch dim.
Each core handles 2048 of the 16384 batch rows.

Host side: u_emb (x1024), w_emb (x32) and -w_emb are concatenated into one
[3V, D] fp8-e4m3 table (neg_w indices offset by +V, pos_w by +2V into the
negated copy, so every score product already carries its loss sign and the
dot reduce needs no sign handling). The gather traffic is descriptor-rate-
bound (~10ns/row/engine), so the layout splits u and w gathers:
  - u-rows are gathered raw fp8 (no cast) and consumed by the TensorEngine,
    which handles fp8 natively;
  - w-rows are gathered with an on-the-fly SWDGE cast to bf16 so the DVE
    keeps its 2x 16-bit rate for the score products.
The scale factors keep fp8 values in the normal range and are divided back
out in the activation's scale factor.

Per-core kernel layout:
  - batch row b -> chunk c = b // 128, partition p = b % 128.
  - 16 chunks in ramped groups. Per group one u-gather (k-major slabs: all
    k=0 rows, then k=1, ...) and one w-gather; w0+w1 share one gather and
    the last group uses a single combined cast gather. Transfer order is
    tuned so the first w block lands right after u0 (DVE starts early).
  - h = sum of the 8 context embeddings: 8 accumulating identity-weight
    matmuls per group (rhs = the k-th fp8 slab) on the TensorEngine ->
    PSUM f32, then one ACT copy PSUM -> SBUF bf16. The last (1-chunk)
    group sums on DVE instead to shorten the tail chain.
  - dots on DVE: one broadcast-mult (2x bf16 rate), 3 contiguous fold-adds
    128->16, one TensorReduce -> softplus(+x) terms for every column.
  - the TensorEngine's HAM activity window is kept hot with filler matmuls
    so the h-sum runs at 2.4 GHz instead of the cold 1.2 GHz.
  - softplus per group on ACT: Exp then Ln(1+x) with accum_out -> lp[:, g].
    Both functions come from one activation table (id 6) preloaded at
    kernel start, so no mid-kernel table swaps.
  - finale: reduce lp rows, PE ones-matmul to collapse partitions -> [1,1],
    host sums the 8 per-core scalars.

loss = sum_b softplus(-score_b) + sum_{b,k} softplus(+neg_score_bk)
"""

import sys

import numpy as np

sys.path.insert(0, "/opt/trn_rl_repo")

from concourse import bacc, bass, mybir, tile  # noqa: E402
from concourse.bass_utils import run_bass_kernel_spmd  # noqa: E402
from concourse.masks import make_identity  # noqa: E402

V, D = 100000, 128
B, C, K = 16384, 8, 5
N_CORES = 8
P = 128
B_LOC = B // N_CORES            # 2048 batch rows per core
N_CHUNK = B_LOC // P            # 16 chunks of 128 rows
GROUPS = (1, 3, 4, 4, 3, 1)     # chunks per gather group; last is combined
assert sum(GROUPS) == N_CHUNK
# matmul moving operand and a PSUM bank both cap at 512 f32 per partition
assert max(GROUPS) * D <= 512
J = 1 + K                       # 6 w-rows per batch row (pos + negs)
R = C + J                       # 14 gathered rows per batch row
NG = len(GROUPS)
NPE = NG - 1                    # groups whose h-sum runs on the PE
STARTS = [sum(GROUPS[:g]) for g in range(NG)]
# gidx column layout: [u0k .. u4k | w0 .. w4 | u5k w5]
U_OFF = [sum(C * n for n in GROUPS[:g]) for g in range(NPE)]
W_BASE = sum(C * n for n in GROUPS[:NPE])
W_OFF = [W_BASE + sum(J * n for n in GROUPS[:g]) for g in range(NPE)]
G5_OFF = W_BASE + sum(J * n for n in GROUPS[:NPE])
assert G5_OFF + R * GROUPS[NPE] == N_CHUNK * R

S_U = 1024.0                    # host-side scale into fp8 normal range
S_W = 32.0
ACT_SCALE = 1.0 / (S_U * S_W)   # divided back out inside the Exp

_NC_CACHE = {}


def _build_bass():
    nc = bacc.Bacc(
        "TRN2",
        target_bir_lowering=False,
        debug=False,
        dynamic_dma_scratch_size=65536,
    )

    bf16 = mybir.dt.bfloat16
    fp32 = mybir.dt.float32
    fp8 = mybir.dt.float8e4
    X = mybir.AxisListType.X
    ADD = mybir.AluOpType.add
    MUL = mybir.AluOpType.mult
    EXP = mybir.ActivationFunctionType.Exp
    LN = mybir.ActivationFunctionType.Ln
    COPY = mybir.ActivationFunctionType.Copy

    emb = nc.dram_tensor("emb_cat", [3 * V, D], fp8, kind="ExternalInput")
    gidx = nc.dram_tensor(
        "gidx", [P, N_CHUNK * R], mybir.dt.int32, kind="ExternalInput"
    )
    loss = nc.dram_tensor("loss_part", [1, 1], fp32, kind="ExternalOutput")

    with tile.TileContext(nc) as tc:
        with (
            tc.tile_pool(name="idx", bufs=1) as idx_pool,
            tc.tile_pool(name="ub", bufs=5) as ub_pool,
            tc.tile_pool(name="wb", bufs=5) as wb_pool,
            tc.tile_pool(name="sc", bufs=2) as sc_pool,
            tc.tile_pool(name="sp", bufs=2) as sp_pool,
            tc.tile_pool(name="fin", bufs=1) as fin_pool,
            tc.tile_pool(name="hp", bufs=3, space="PSUM") as hp_pool,
            tc.tile_pool(name="ps", bufs=1, space="PSUM") as ps_pool,
        ):
            # one table (id 6: natural_log_exp_and_others) serves Exp AND Ln
            nc.scalar.add_instruction(mybir.InstLoadActFuncSet(
                name=nc.get_next_instruction_name(), act_func_set_id=6,
                ins=[], outs=[]))

            ix_all = idx_pool.tile([P, N_CHUNK * R], mybir.dt.int32, tag="ix")
            n0 = C * GROUPS[0]   # u0 columns only: first desc-gen starts ASAP
            nc.sync.dma_start(out=ix_all[:, 0:n0], in_=gidx[:, 0:n0])
            nc.sync.dma_start(out=ix_all[:, n0:], in_=gidx[:, n0:])

            ident = fin_pool.tile([P, P], fp8, tag="ident")
            make_identity(nc, ident[:])
            ones = fin_pool.tile([P, 1], fp32, tag="ones")
            nc.gpsimd.memset(ones[:], 1.0)

            warm = ps_pool.tile([P, P], fp32, tag="warm")

            def pe_filler(count):
                # keep the TensorEngine's activity window hot; results unused
                for _ in range(count):
                    nc.tensor.matmul(out=warm[:], lhsT=ident[:],
                                     rhs=ident[:], start=True, stop=True)

            pe_filler(45)

            h_sb = fin_pool.tile([P, N_CHUNK * D], bf16, tag="h_sb")
            h16 = h_sb[:].rearrange("p (c d) -> p c d", c=N_CHUNK)
            lp = fin_pool.tile([P, NG], fp32, tag="lp")

            def gather(cols, ncols, dtype, tag, pool):
                t = pool.tile([P, ncols * D], dtype, tag=tag)
                nc.gpsimd.indirect_dma_start(
                    out=t[:],
                    out_offset=None,
                    in_=emb[:],
                    in_offset=bass.IndirectOffsetOnAxis(
                        ap=ix_all[:, cols : cols + ncols], axis=0
                    ),
                )
                return t

            # issue all gathers up front; the list order is the transfer
            # order: u0 | w0+u5+w5 (merged cast gather; the tail group's
            # data comes early since its DVE work is emitted right after
            # g0's to fill the pipeline-fill bubble) | u1 | w1 | ... | w4
            ub, wbt = {}, {}
            n5 = GROUPS[NPE]
            ub[0] = gather(U_OFF[0], C * GROUPS[0], fp8, "ub", ub_pool)
            w01 = gather(W_OFF[0], J * (GROUPS[0] + GROUPS[1]), bf16, "wb",
                         wb_pool)
            wbt[0] = w01[:, 0 : J * GROUPS[0] * D]
            wbt[1] = w01[:, J * GROUPS[0] * D :]
            m0 = gather(G5_OFF, R * n5, bf16, "g5", wb_pool)
            ub[NPE] = m0[:, 0 : C * n5 * D]
            wbt[NPE] = m0[:, C * n5 * D :]
            ub[1] = gather(U_OFF[1], C * GROUPS[1], fp8, "ub", ub_pool)
            for _g in range(2, NPE):
                ub[_g] = gather(U_OFF[_g], C * GROUPS[_g], fp8, "ub", ub_pool)
                wbt[_g] = gather(W_OFF[_g], J * GROUPS[_g], bf16, "wb",
                                 wb_pool)[:]

            for g in [0, NG - 1] + list(range(1, NPE)):
                n = GROUPS[g]
                c0 = STARTS[g]
                if g < NPE:
                    # h-sum on the TensorEngine: 8 accumulating identity
                    # matmuls (rhs = the k-th fp8 u-slab) -> PSUM f32
                    hp = hp_pool.tile([P, n * D], fp32, tag="hp")
                    for k in range(C):
                        nc.tensor.matmul(
                            out=hp[:],
                            lhsT=ident[:],
                            rhs=ub[g][:, k * n * D : (k + 1) * n * D],
                            start=(k == 0),
                            stop=(k == C - 1),
                        )
                    # PSUM f32 -> SBUF bf16 on the Scalar engine
                    nc.scalar.activation(
                        out=h_sb[:, c0 * D : (c0 + n) * D], in_=hp[:],
                        func=COPY,
                    )
                    if g + 1 < NPE:
                        pe_filler(30)
                else:
                    # tail group: binary-tree h-sum on DVE over the k-major
                    # bf16 slabs, last fold writes straight into h_sb
                    u5 = ub[g]
                    nc.vector.tensor_add(
                        out=u5[:, 0 : 4 * n * D], in0=u5[:, 0 : 4 * n * D],
                        in1=u5[:, 4 * n * D : 8 * n * D],
                    )
                    nc.vector.tensor_add(
                        out=u5[:, 0 : 2 * n * D], in0=u5[:, 0 : 2 * n * D],
                        in1=u5[:, 2 * n * D : 4 * n * D],
                    )
                    nc.vector.tensor_add(
                        out=h_sb[:, c0 * D : (c0 + n) * D],
                        in0=u5[:, 0 : n * D], in1=u5[:, n * D : 2 * n * D],
                    )

                # m[p, c, j, d] = w[p, c, j, d] * h[p, c, d], in place over w
                w4 = wbt[g].rearrange("p (c j d) -> p c j d", c=n, j=J)
                h4 = h16[:, c0 : c0 + n]
                nc.vector.tensor_mul(
                    out=w4,
                    in0=w4,
                    in1=h4[:, :, None, :].broadcast_to([P, n, J, D]),
                )
                # fold the innermost 128 -> 16 with bf16 adds before the
                # (1x-rate) TensorReduce
                for w_ in (64, 32, 16):
                    nc.vector.tensor_add(
                        out=w4[:, :, :, 0:w_],
                        in0=w4[:, :, :, 0:w_],
                        in1=w4[:, :, :, w_ : 2 * w_],
                    )
                # raw dots (f32); j=0 was sign-folded in the multiply
                sc = sc_pool.tile([P, n * J], fp32, tag="sc")
                sc3 = sc[:].rearrange("p (c j) -> p c j", j=J)
                nc.vector.tensor_reduce(
                    out=sc3, in_=w4[:, :, :, 0:16], axis=X, op=ADD,
                )
                # softplus(x) = ln(1 + exp(x)); accumulate into lp[:, g]
                sp = sp_pool.tile([P, n * J], fp32, tag="sp")
                nc.scalar.activation(out=sp[:], in_=sc[:], func=EXP,
                                     scale=ACT_SCALE)
                nc.scalar.activation(out=sp[:], in_=sp[:], func=LN, bias=1.0,
                                     accum_out=lp[:, g : g + 1])

            # per-partition loss, then collapse partitions via ones-matmul
            lp1 = fin_pool.tile([P, 1], fp32, tag="lp1")
            nc.vector.tensor_reduce(out=lp1[:], in_=lp[:], axis=X, op=ADD)
            acc = ps_pool.tile([1, 1], fp32)
            nc.tensor.matmul(out=acc[:], lhsT=ones[:], rhs=lp1[:],
                             start=True, stop=True)
            out_sb = fin_pool.tile([1, 1], fp32, tag="out")
            nc.vector.tensor_copy(out=out_sb[:], in_=acc[:])
            nc.sync.dma_start(out=loss[:], in_=out_sb[:])

    nc.compile()
    return nc


def _get_nc():
    if "nc" not in _NC_CACHE:
        _NC_CACHE["nc"] = _build_bass()
    return _NC_CACHE["nc"]


def _make_in_maps(pos_u, pos_w, neg_w, u_emb, w_emb):
    pos_u = np.asarray(pos_u).astype(np.int32)
    pos_w = np.asarray(pos_w).astype(np.int32)
    neg_w = np.asarray(neg_w).astype(np.int32)
    u_emb = np.asarray(u_emb, dtype=np.float32)
    w_emb = np.asarray(w_emb, dtype=np.float32)

    fp8_np = mybir.dt.np(mybir.dt.float8e4)
    w_s = np.clip(w_emb * S_W, -224.0, 224.0)
    emb_cat = np.ascontiguousarray(
        np.concatenate(
            [np.clip(u_emb * S_U, -224.0, 224.0), w_s, -w_s], axis=0
        ).astype(fp8_np)
    )

    in_maps = []
    for i in range(N_CORES):
        sl = slice(i * B_LOC, (i + 1) * B_LOC)
        # batch row b -> (chunk c = b // 128, partition p = b % 128)
        rows_u = pos_u[sl].reshape(N_CHUNK, P, C)          # [c, p, k]
        rows_w = np.concatenate(
            [pos_w[sl, None] + 2 * V, neg_w[sl] + V], axis=1
        ).reshape(N_CHUNK, P, J)                           # [c, p, j]
        # columns: [u0k .. u4k | w0 .. w4 | u5k w5]; u blocks k-major
        cols = []
        for g in range(NPE):
            c0, n = STARTS[g], GROUPS[g]
            cols.append(rows_u[c0 : c0 + n].transpose(1, 2, 0).reshape(P, -1))
        for g in range(NPE):
            c0, n = STARTS[g], GROUPS[g]
            cols.append(rows_w[c0 : c0 + n].transpose(1, 0, 2).reshape(P, -1))
        c0, n = STARTS[NPE], GROUPS[NPE]
        cols.append(rows_u[c0 : c0 + n].transpose(1, 2, 0).reshape(P, -1))
        cols.append(rows_w[c0 : c0 + n].transpose(1, 0, 2).reshape(P, -1))
        gidx = np.concatenate(cols, axis=1)                # [P, N_CHUNK * R]
        in_maps.append(
            {
                "emb_cat": emb_cat,
                "gidx": np.ascontiguousarray(gidx),
            }
        )
    return in_maps


def _install_axon_profile_shim():
    """Provide antenv.axon_hooks (missing in this image) so trace=True can
    capture NTFF profiles via the axon PJRT .so, and keep trace artifacts
    local instead of uploading to a bucket."""
    import contextlib
    import ctypes
    import types

    import concourse.bass_utils as bu

    bu.upload_artifacts = lambda tmpdir: tmpdir

    try:
        from antenv.axon_hooks import get_axon_ntff_profile_hook  # noqa: F401

        return
    except ImportError:
        pass

    mod = types.ModuleType("antenv.axon_hooks")
    holder = {}
    mod.set_axon_ntff_profile_hook = lambda h: holder.__setitem__("h", h)
    mod.get_axon_ntff_profile_hook = lambda: holder.get("h")
    sys.modules["antenv.axon_hooks"] = mod
    import antenv

    antenv.axon_hooks = mod

    so_path = "/opt/axon/libaxon_pjrt.so"
    lib = ctypes.CDLL(so_path)
    if not hasattr(lib, "axon_start_nrt_profile"):
        return
    lib.axon_start_nrt_profile.argtypes = [
        ctypes.POINTER(ctypes.c_int64),
        ctypes.c_size_t,
    ]
    lib.axon_start_nrt_profile.restype = ctypes.c_int64
    lib.axon_stop_nrt_profile.argtypes = [ctypes.c_char_p]
    lib.axon_stop_nrt_profile.restype = ctypes.c_int64

    @contextlib.contextmanager
    def _hook(output_dir, device_ids):
        import jax

        jax.devices()
        if device_ids:
            ids = (ctypes.c_int64 * len(device_ids))(*device_ids)
            rc = lib.axon_start_nrt_profile(ids, len(device_ids))
        else:
            rc = lib.axon_start_nrt_profile(None, 0)
        if rc != 0:
            raise RuntimeError(f"axon_start_nrt_profile rc={rc}")
        try:
            yield
        finally:
            n = lib.axon_stop_nrt_profile(str(output_dir).encode())
            print(f"profile: {n} file(s) written to {output_dir}")

    mod.set_axon_ntff_profile_hook(_hook)


def _run(in_maps, trace=False):
    if trace:
        _install_axon_profile_shim()
    nc = _get_nc()
    return run_bass_kernel_spmd(nc, in_maps, list(range(N_CORES)), trace=trace)


def kernel(pos_u, pos_w, neg_w, u_emb, w_emb):
    in_maps = _make_in_maps(pos_u, pos_w, neg_w, u_emb, w_emb)
    bkr = _run(in_maps, trace=False)
    total = 0.0
    for r in bkr.results:
        total += float(r["loss_part"].astype(np.float64).sum())
    return np.float32(total)


def kernel_traced(pos_u, pos_w, neg_w, u_emb, w_emb):
    """Like kernel() but returns (loss, BassKernelResults) with HW profile."""
    in_maps = _make_in_maps(pos_u, pos_w, neg_w, u_emb, w_emb)
    bkr = _run(in_maps, trace=True)
    total = 0.0
    for r in bkr.results:
        total += float(r["loss_part"].astype(np.float64).sum())
    return np.float32(total), bkr
